# revision 11
# baseline (speedup 1.0000x reference)
"""APPNP GNN (GCN-normalized propagation, K=5, alpha=0.5) on 8 TRN2 NeuronCores.

Strategy:
  - Nodes relabeled: dealt round-robin by degree to 8 cores (balanced shards),
    then ordered within each core by max-per-range in-edge count so that each
    128-dest partition-block has near-uniform per-range slot needs.
  - h staged as a bf16 [100352, 128] table replicated in each core's DRAM,
    re-assembled per hop via AllGather of the 8 shard updates (one Shared
    table per hop: single-writer rule).
  - Per hop each core gathers its edges' source rows with gpsimd.dma_gather
    (int16 indices into 25088-row windows = core-pair ranges; dest-slot
    layout: partition = dest-in-block, column = (range, slot j, block)),
    accumulating slots into a bf16 accumulator on VectorE.
  - GCN norm folded into per-node scalings: table rows are dinv*h and the
    per-dest dinv folds into A = 0.5*dinv^2 at the blend:
        next_table = A * acc + 0.5*dinv*h0
  - lin1/lin2 run on TensorE in bf16.
"""

import os
import sys
import types

import numpy as np

sys.path.insert(0, "/opt/trn_rl_repo")


def _install_ntff_shim():
    """antenv.axon_hooks is absent in this image; concourse imports it when
    trace=True under axon. Provide it, backed by the boot's ctypes hook."""
    if "antenv.axon_hooks" in sys.modules:
        return
    mod = types.ModuleType("antenv.axon_hooks")
    mod._hook = None

    def set_axon_ntff_profile_hook(h):
        mod._hook = h

    def get_axon_ntff_profile_hook():
        if mod._hook is None:
            try:
                from trn_agent_boot.trn_boot import _ntff_profile_via_ctypes

                mod._hook = _ntff_profile_via_ctypes("/opt/axon/libaxon_pjrt.so")
            except Exception:
                return None
        return mod._hook

    mod.set_axon_ntff_profile_hook = set_axon_ntff_profile_hook
    mod.get_axon_ntff_profile_hook = get_axon_ntff_profile_hook
    sys.modules["antenv.axon_hooks"] = mod


_install_ntff_shim()

import ml_dtypes  # noqa: E402

import concourse.tile as tile  # noqa: E402
from concourse import bacc, mybir  # noqa: E402
from concourse.bass_utils import run_bass_kernel_spmd  # noqa: E402

N = 100000
NC = 8
C = 128
INC = 512
OUTC = 40
K = 5
NB = 98
SH = NB * 128          # 12544 dest slots per core
NPAD = NC * SH         # 100352
# position 12500 of core 0 is a pad node (dinv=0): its table row is always 0,
# so it doubles as the zero row for padded gather slots.
ZROW = 12500
GW = 98                # gather columns per vals tile

BF = mybir.dt.bfloat16
F32 = mybir.dt.float32
I32 = mybir.dt.int32

DT_NS = None


def _bf16(a):
    return np.ascontiguousarray(a.astype(ml_dtypes.bfloat16))


def _preprocess(x, edge_index, W1, b1, W2, b2):
    x = np.asarray(x, dtype=np.float32)
    ei = np.asarray(edge_index).astype(np.int64)
    W1 = np.asarray(W1, dtype=np.float32)
    b1 = np.asarray(b1, dtype=np.float32)
    W2 = np.asarray(W2, dtype=np.float32)
    b2 = np.asarray(b2, dtype=np.float32)

    loops = np.arange(N, dtype=np.int64)
    row = np.concatenate([ei[0], loops])
    col = np.concatenate([ei[1], loops])
    deg = np.bincount(col, minlength=N).astype(np.int64)
    dinv = np.where(deg > 0, 1.0 / np.sqrt(deg.astype(np.float64)), 0.0)

    # relabel: deal nodes round-robin by degree to the 8 cores, positions by
    # degree desc within each core (block slot-counts then decrease with b)
    order = np.argsort(-deg, kind="stable")
    i = np.arange(N)
    newid = np.empty(N, dtype=np.int64)
    newid[order] = (i % NC) * SH + (i // NC)

    rn = newid[row]
    cn = newid[col]

    # slot index j within each destination
    s = np.argsort(cn, kind="stable")
    cn_s = cn[s]
    rn_s = rn[s]
    cnt = np.bincount(cn_s, minlength=NC * SH)
    starts = np.zeros(NC * SH, dtype=np.int64)
    np.cumsum(cnt[:-1], out=starts[1:])
    j_s = np.arange(len(cn_s)) - starts[cn_s]

    # unified per-block max degree across cores, monotone over blocks
    Lb = cnt.reshape(NC, NB, 128).max(axis=(0, 2)).astype(np.int64)
    Lb = np.maximum.accumulate(Lb[::-1])[::-1]
    maxL = int(Lb.max())
    nb_j = np.array([(Lb > j).sum() for j in range(maxL)], dtype=np.int64)
    off = np.zeros(maxL + 1, dtype=np.int64)
    np.cumsum(nb_j, out=off[1:])
    TOTW = int(off[-1])

    # idx tensors [NC, 128, TOTW] int32 (global row ids)
    idx = np.full((NC, 128, TOTW), ZROW, dtype=np.int32)
    core_s = cn_s // SH
    dloc = cn_s % SH
    b_s = dloc // 128
    p_s = dloc % 128
    colpos = off[j_s] + b_s
    idx[core_s, p_s, colpos] = rn_s.astype(np.int32)

    # column groups of <=GW with per-j acc-add segments
    groups = []  # (c0, c1, [(j, b0, b1, col_in_group)])
    c0 = 0
    while c0 < TOTW:
        c1 = min(c0 + GW, TOTW)
        segs = []
        for j in range(maxL):
            lo = max(int(off[j]), c0)
            hi = min(int(off[j + 1]), c1)
            if lo < hi:
                segs.append((j, lo - int(off[j]), hi - int(off[j]), lo - c0))
        groups.append((c0, c1, segs))
        c0 = c1
    meta = dict(TOTW=TOTW, groups=groups)

    # per-core node data in new-id space
    xfull = np.zeros((NC * SH, INC), dtype=np.float32)
    xfull[newid] = x
    dinvN = np.zeros(NC * SH, dtype=np.float32)
    dinvN[newid] = dinv.astype(np.float32)

    in_maps = []
    w1_bf = _bf16(W1)
    b1r = np.ascontiguousarray(np.tile(b1[None, :], (128, 4)).astype(np.float32))
    w2_bf = _bf16(W2)
    b2r = np.ascontiguousarray(np.tile(b2[None, :], (128, 1)).astype(np.float32))
    for c in range(NC):
        dv = dinvN[c * SH : (c + 1) * SH].reshape(NB, 128).T  # [128, NB]
        a_pb = (0.5 * dv * dv).astype(np.float32)
        dh_pb = (0.5 * dv).astype(np.float32)
        a_exp = np.repeat(a_pb[:, :, None], C, axis=2).reshape(128, NB * C)
        dinvh_exp = np.repeat(dh_pb[:, :, None], C, axis=2).reshape(128, NB * C)
        dinv_exp = np.repeat(dv[:, :, None].astype(np.float32), C, axis=2).reshape(
            128, NB * C
        )
        in_maps.append(
            {
                "xt": _bf16(xfull[c * SH : (c + 1) * SH].T),
                "idx": np.ascontiguousarray(idx[c]),
                "a_exp": _bf16(a_exp),
                "dinvh_exp": _bf16(dinvh_exp),
                "dinv_exp": _bf16(dinv_exp),
                "w1": w1_bf,
                "b1r": b1r,
                "w2": w2_bf,
                "b2r": b2r,
            }
        )
    return in_maps, newid, meta


def _build(meta):
    TOTW = meta["TOTW"]
    groups = meta["groups"]

    nc = bacc.Bacc("TRN2", target_bir_lowering=False, debug=False, num_devices=NC)
    xt_d = nc.dram_tensor("xt", [INC, SH], BF, kind="ExternalInput").ap()
    idx_d = nc.dram_tensor("idx", [128, TOTW], I32, kind="ExternalInput").ap()
    a_d = nc.dram_tensor("a_exp", [128, SH], BF, kind="ExternalInput").ap()
    dinvh_d = nc.dram_tensor("dinvh_exp", [128, SH], BF, kind="ExternalInput").ap()
    dinve_d = nc.dram_tensor("dinv_exp", [128, SH], BF, kind="ExternalInput").ap()
    w1_d = nc.dram_tensor("w1", [INC, C], BF, kind="ExternalInput").ap()
    b1r_d = nc.dram_tensor("b1r", [128, 4 * C], F32, kind="ExternalInput").ap()
    w2_d = nc.dram_tensor("w2", [C, OUTC], BF, kind="ExternalInput").ap()
    b2r_d = nc.dram_tensor("b2r", [128, OUTC], F32, kind="ExternalInput").ap()
    out_d = nc.dram_tensor("out", [SH, OUTC], F32, kind="ExternalOutput").ap()

    with tile.TileContext(nc) as tc:
        with (
            tc.tile_pool(name="dram", bufs=1, space="DRAM") as dramp,
            tc.tile_pool(name="perm", bufs=1) as perm,
        ):
            tables = []
            for kk in range(K):
                tab_k = dramp.tile(
                    [NPAD, C], BF, addr_space="Shared", name=f"table{kk}"
                )
                tables.append(tab_k)
            agin_d = dramp.tile([SH, C], BF)
            x0h_d = dramp.tile([128, SH], F32)

            acc = perm.tile([128, SH], BF)
            agin_sb = perm.tile([128, SH], BF)

            with tc.tile_pool(name="hop", bufs=1) as hop:
                a_sb = hop.tile([128, SH], BF)
                nc.sync.dma_start(out=a_sb[:], in_=a_d[:, :])
                x0d_sb = hop.tile([128, SH], BF)

                # ---------------- phase 1: h0 = x @ W1 + b1 ----------------
                with (
                    tc.tile_pool(name="p1", bufs=2) as p1,
                    tc.tile_pool(name="p1c", bufs=1) as p1c,
                    tc.tile_pool(name="ps1", bufs=2, space="PSUM") as ps1,
                ):
                    w1sb = p1c.tile([128, INC // 128, C], BF)
                    nc.sync.dma_start(
                        out=w1sb[:],
                        in_=w1_d.rearrange("(k p) c -> p k c", p=128),
                    )
                    b1r_sb = p1c.tile([128, 4 * C], F32)
                    nc.sync.dma_start(out=b1r_sb[:], in_=b1r_d[:, :])
                    dinvh1_sb = p1c.tile([128, SH], BF)
                    nc.sync.dma_start(out=dinvh1_sb[:], in_=dinvh_d[:, :])
                    dinve1_sb = p1c.tile([128, SH], BF)
                    nc.sync.dma_start(out=dinve1_sb[:], in_=dinve_d[:, :])

                    NGRP = (NB + 3) // 4
                    for g in range(NGRP):
                        nb4 = min(4, NB - 4 * g)
                        w = nb4 * 128
                        xts_list = []
                        for kc in range(4):
                            xts = p1.tile([128, 512], BF, tag=f"xts{kc}")
                            nc.sync.dma_start(
                                out=xts[:, :w],
                                in_=xt_d[
                                    kc * 128 : (kc + 1) * 128,
                                    g * 512 : g * 512 + w,
                                ],
                            )
                            xts_list.append(xts)
                        h0g = p1.tile([128, 512], F32, tag="h0g")
                        for bb in range(nb4):
                            psum = ps1.tile([128, 128], F32, tag=f"ps{bb}")
                            for kc in range(4):
                                nc.tensor.matmul(
                                    out=psum[:],
                                    lhsT=xts_list[kc][:, bb * 128 : (bb + 1) * 128],
                                    rhs=w1sb[:, kc, :],
                                    start=(kc == 0),
                                    stop=(kc == 3),
                                )
                            nc.vector.tensor_add(
                                h0g[:, bb * 128 : (bb + 1) * 128],
                                psum[:],
                                b1r_sb[:, bb * 128 : (bb + 1) * 128],
                            )
                        sl = slice(g * 512, g * 512 + w)
                        # x0h = 0.5*h0 (f32, to DRAM for the k=K blend)
                        x0hg = p1.tile([128, 512], F32, tag="x0hg")
                        nc.scalar.mul(x0hg[:, :w], h0g[:, :w], 0.5)
                        nc.sync.dma_start(out=x0h_d[:, sl], in_=x0hg[:, :w])
                        # x0d = 0.5*dinv*h0 (bf16, resident)
                        nc.vector.tensor_mul(
                            x0d_sb[:, sl], h0g[:, :w], dinvh1_sb[:, sl]
                        )
                        # ag0 = dinv*h0 (bf16)
                        nc.vector.tensor_mul(
                            agin_sb[:, sl], h0g[:, :w], dinve1_sb[:, sl]
                        )

                # ---------------- hops ----------------
                agin_rows = agin_d[:].rearrange("(b p) c -> p b c", p=128)
                agin_view = agin_sb[:].rearrange("p (b c) -> p b c", c=C)
                with (
                    tc.tile_pool(name="vals", bufs=3) as valsp,
                    tc.tile_pool(name="idxp", bufs=1) as idxp,
                ):
                    idx_sb = idxp.tile([128, TOTW], I32)
                    nc.sync.dma_start(out=idx_sb[:], in_=idx_d[:, :])
                    for k in range(1, K + 1):
                        nc.sync.dma_start(out=agin_rows, in_=agin_view)
                        table = tables[k - 1]
                        nc.gpsimd.collective_compute(
                            "AllGather",
                            mybir.AluOpType.bypass,
                            replica_groups=[list(range(NC))],
                            ins=[agin_d[:].opt()],
                            outs=[table[:, :].opt()],
                        )
                        nc.vector.memset(acc[:], 0.0)
                        from concourse.bass import IndirectOffsetOnAxis
                        for c0g, c1g, segs in groups:
                            gwid = c1g - c0g
                            vals = valsp.tile([128, GW * C], BF, tag="vals")
                            for kk in range(gwid):
                                nc.gpsimd.indirect_dma_start(
                                    out=vals[:, kk * C : (kk + 1) * C],
                                    out_offset=None,
                                    in_=table[:],
                                    in_offset=IndirectOffsetOnAxis(
                                        ap=idx_sb[:, c0g + kk : c0g + kk + 1],
                                        axis=0,
                                    ),
                                )
                            for jj, b0, b1_, cg in segs:
                                src_ap = vals[:, cg * C : (cg + b1_ - b0) * C]
                                dst = acc[:, b0 * C : b1_ * C]
                                nc.vector.tensor_add(dst, dst, src_ap)
                        if k < K:
                            # next table shard = A*acc + x0d
                            nc.vector.tensor_mul(agin_sb[:], acc[:], a_sb[:])
                            nc.vector.tensor_add(agin_sb[:], agin_sb[:], x0d_sb[:])

            # ---------------- phase 3: out = h5 @ W2 + b2 ----------------
            with (
                tc.tile_pool(name="p3", bufs=2) as p3,
                tc.tile_pool(name="p3c", bufs=1) as p3c,
                tc.tile_pool(name="ps3", bufs=4, space="PSUM") as ps3,
            ):
                x0h_sb = p3c.tile([128, SH], F32)
                nc.sync.dma_start(out=x0h_sb[:], in_=x0h_d[:, :])
                dinvh3_sb = p3c.tile([128, SH], BF)
                nc.sync.dma_start(out=dinvh3_sb[:], in_=dinvh_d[:, :])
                w2sb = p3c.tile([C, OUTC], BF)
                nc.sync.dma_start(out=w2sb[:], in_=w2_d[:, :])
                b2r_sb = p3c.tile([128, OUTC], F32)
                nc.sync.dma_start(out=b2r_sb[:], in_=b2r_d[:, :])
                ident = p3c.tile([128, 128], BF)
                from concourse.masks import make_identity

                make_identity(nc, ident[:])
                out_sb = p3c.tile([128, NB * OUTC], F32)

                # h5 = 0.5*dinv*acc + 0.5*h0  (bf16, into agin_sb)
                nc.vector.tensor_mul(agin_sb[:], acc[:], dinvh3_sb[:])
                nc.vector.tensor_add(agin_sb[:], agin_sb[:], x0h_sb[:])

                for b in range(NB):
                    pst = ps3.tile([128, 128], BF, tag="pst")
                    nc.tensor.transpose(
                        out=pst[:],
                        in_=agin_sb[:, b * C : (b + 1) * C],
                        identity=ident[:],
                    )
                    h5t = p3.tile([128, 128], BF, tag="h5t")
                    nc.vector.tensor_copy(out=h5t[:], in_=pst[:])
                    pso = ps3.tile([128, OUTC], F32, tag="pso")
                    nc.tensor.matmul(
                        out=pso[:],
                        lhsT=h5t[:],
                        rhs=w2sb[:],
                        start=True,
                        stop=True,
                    )
                    nc.vector.tensor_add(
                        out_sb[:, b * OUTC : (b + 1) * OUTC], pso[:], b2r_sb[:]
                    )
                nc.sync.dma_start(
                    out=out_d.rearrange("(b p) o -> p b o", p=128),
                    in_=out_sb[:].rearrange("p (b o) -> p b o", o=OUTC),
                )

    nc.compile()
    return nc


def _run(inputs, trace=False):
    in_maps, newid, meta = _preprocess(**inputs)
    nc = _build(meta)
    res = run_bass_kernel_spmd(
        nc, in_maps, core_ids=list(range(NC)), trace=trace
    )
    full = np.concatenate([res.results[c]["out"] for c in range(NC)], axis=0)
    out = full[newid].astype(np.float32)
    return out, res.exec_time_ns


def kernel(**inputs) -> np.ndarray:
    trace = bool(int(os.environ.get("APPNP_TRACE", "0")))
    out, _ = _run(inputs, trace=trace)
    return out


if __name__ == "__main__":
    rng = np.random.default_rng(0)
    demo = {
        "x": rng.standard_normal((N, INC), dtype=np.float32),
        "edge_index": rng.integers(0, N, size=(2, 1600000)).astype(np.int64),
        "W1": rng.standard_normal((INC, C), dtype=np.float32) / 22.6,
        "b1": rng.standard_normal(C, dtype=np.float32) * 0.01,
        "W2": rng.standard_normal((C, OUTC), dtype=np.float32) / 11.3,
        "b2": rng.standard_normal(OUTC, dtype=np.float32) * 0.01,
    }
    out, ns = _run(demo, trace=True)
    print("exec_time_ns:", ns, "out", out.shape, out.dtype)


# revision 12
# speedup vs baseline: 1.1653x; 1.1653x over previous
"""APPNP GNN (GCN-normalized propagation, K=5, alpha=0.5) on 8 TRN2 NeuronCores.

Strategy:
  - Nodes relabeled: dealt round-robin by degree to 8 cores (balanced shards),
    then ordered within each core by max-per-range in-edge count so that each
    128-dest partition-block has near-uniform per-range slot needs.
  - h staged as a bf16 [100352, 128] table replicated in each core's DRAM,
    re-assembled per hop via AllGather of the 8 shard updates (one Shared
    table per hop: single-writer rule).
  - Per hop each core gathers its edges' source rows with gpsimd.dma_gather
    (int16 indices into 25088-row windows = core-pair ranges; dest-slot
    layout: partition = dest-in-block, column = (range, slot j, block)),
    accumulating slots into a bf16 accumulator on VectorE.
  - GCN norm folded into per-node scalings: table rows are dinv*h and the
    per-dest dinv folds into A = 0.5*dinv^2 at the blend:
        next_table = A * acc + 0.5*dinv*h0
  - lin1/lin2 run on TensorE in bf16.
"""

import os
import sys
import types

import numpy as np

sys.path.insert(0, "/opt/trn_rl_repo")


def _install_ntff_shim():
    """antenv.axon_hooks is absent in this image; concourse imports it when
    trace=True under axon. Provide it, backed by the boot's ctypes hook."""
    if "antenv.axon_hooks" in sys.modules:
        return
    mod = types.ModuleType("antenv.axon_hooks")
    mod._hook = None

    def set_axon_ntff_profile_hook(h):
        mod._hook = h

    def get_axon_ntff_profile_hook():
        if mod._hook is None:
            try:
                from trn_agent_boot.trn_boot import _ntff_profile_via_ctypes

                mod._hook = _ntff_profile_via_ctypes("/opt/axon/libaxon_pjrt.so")
            except Exception:
                return None
        return mod._hook

    mod.set_axon_ntff_profile_hook = set_axon_ntff_profile_hook
    mod.get_axon_ntff_profile_hook = get_axon_ntff_profile_hook
    sys.modules["antenv.axon_hooks"] = mod


_install_ntff_shim()

import ml_dtypes  # noqa: E402

import concourse.tile as tile  # noqa: E402
from concourse import bacc, mybir  # noqa: E402
from concourse.bass_utils import run_bass_kernel_spmd  # noqa: E402

N = 100000
NC = 8
C = 128
INC = 512
OUTC = 40
K = 5
NB = 98
SH = NB * 128          # 12544 dest slots per core
NPAD = NC * SH         # 100352
# position 12500 of core 0 is a pad node (dinv=0): its table row is always 0,
# so it doubles as the zero row for padded gather slots.
ZROW = 12500
GW = 98                # gather columns per vals tile

BF = mybir.dt.bfloat16
F32 = mybir.dt.float32
I32 = mybir.dt.int32

DT_NS = None


def _bf16(a):
    return np.ascontiguousarray(a.astype(ml_dtypes.bfloat16))


def _preprocess(x, edge_index, W1, b1, W2, b2):
    x = np.asarray(x, dtype=np.float32)
    ei = np.asarray(edge_index).astype(np.int64)
    W1 = np.asarray(W1, dtype=np.float32)
    b1 = np.asarray(b1, dtype=np.float32)
    W2 = np.asarray(W2, dtype=np.float32)
    b2 = np.asarray(b2, dtype=np.float32)

    loops = np.arange(N, dtype=np.int64)
    row = np.concatenate([ei[0], loops])
    col = np.concatenate([ei[1], loops])
    deg = np.bincount(col, minlength=N).astype(np.int64)
    dinv = np.where(deg > 0, 1.0 / np.sqrt(deg.astype(np.float64)), 0.0)

    # relabel: deal nodes round-robin by degree to the 8 cores, positions by
    # degree desc within each core (block slot-counts then decrease with b)
    order = np.argsort(-deg, kind="stable")
    i = np.arange(N)
    newid = np.empty(N, dtype=np.int64)
    newid[order] = (i % NC) * SH + (i // NC)

    rn = newid[row]
    cn = newid[col]

    # slot index j within each destination
    s = np.argsort(cn, kind="stable")
    cn_s = cn[s]
    rn_s = rn[s]
    cnt = np.bincount(cn_s, minlength=NC * SH)
    starts = np.zeros(NC * SH, dtype=np.int64)
    np.cumsum(cnt[:-1], out=starts[1:])
    j_s = np.arange(len(cn_s)) - starts[cn_s]

    # unified per-block max degree across cores, monotone over blocks
    Lb = cnt.reshape(NC, NB, 128).max(axis=(0, 2)).astype(np.int64)
    Lb = np.maximum.accumulate(Lb[::-1])[::-1]
    maxL = int(Lb.max())
    nb_j = np.array([(Lb > j).sum() for j in range(maxL)], dtype=np.int64)
    off = np.zeros(maxL + 1, dtype=np.int64)
    np.cumsum(nb_j, out=off[1:])
    TOTW = int(off[-1])

    # idx tensors [NC, 128, TOTW] int32 (global row ids)
    idx = np.full((NC, 128, TOTW), ZROW, dtype=np.int32)
    core_s = cn_s // SH
    dloc = cn_s % SH
    b_s = dloc // 128
    p_s = dloc % 128
    colpos = off[j_s] + b_s
    idx[core_s, p_s, colpos] = rn_s.astype(np.int32)

    # column groups of <=GW with per-j acc-add segments
    groups = []  # (c0, c1, [(j, b0, b1, col_in_group)])
    c0 = 0
    while c0 < TOTW:
        c1 = min(c0 + GW, TOTW)
        segs = []
        for j in range(maxL):
            lo = max(int(off[j]), c0)
            hi = min(int(off[j + 1]), c1)
            if lo < hi:
                segs.append((j, lo - int(off[j]), hi - int(off[j]), lo - c0))
        groups.append((c0, c1, segs))
        c0 = c1
    meta = dict(TOTW=TOTW, groups=groups)

    # per-core node data in new-id space
    xfull = np.zeros((NC * SH, INC), dtype=np.float32)
    xfull[newid] = x
    dinvN = np.zeros(NC * SH, dtype=np.float32)
    dinvN[newid] = dinv.astype(np.float32)

    in_maps = []
    w1_bf = _bf16(W1)
    b1r = np.ascontiguousarray(np.tile(b1[None, :], (128, 4)).astype(np.float32))
    w2_bf = _bf16(W2)
    b2r = np.ascontiguousarray(np.tile(b2[None, :], (128, 1)).astype(np.float32))
    for c in range(NC):
        dv = dinvN[c * SH : (c + 1) * SH].reshape(NB, 128).T  # [128, NB]
        a_pb = (0.5 * dv * dv).astype(np.float32)
        dh_pb = (0.5 * dv).astype(np.float32)
        a_exp = np.repeat(a_pb[:, :, None], C, axis=2).reshape(128, NB * C)
        dinvh_exp = np.repeat(dh_pb[:, :, None], C, axis=2).reshape(128, NB * C)
        dinv_exp = np.repeat(dv[:, :, None].astype(np.float32), C, axis=2).reshape(
            128, NB * C
        )
        in_maps.append(
            {
                "xt": _bf16(xfull[c * SH : (c + 1) * SH].T),
                "idx": np.ascontiguousarray(idx[c]),
                "a_exp": _bf16(a_exp),
                "dinvh_exp": _bf16(dinvh_exp),
                "dinv_exp": _bf16(dinv_exp),
                "w1": w1_bf,
                "b1r": b1r,
                "w2": w2_bf,
                "b2r": b2r,
            }
        )
    return in_maps, newid, meta


def _build(meta):
    TOTW = meta["TOTW"]
    groups = meta["groups"]

    nc = bacc.Bacc("TRN2", target_bir_lowering=False, debug=False, num_devices=NC,
                   num_swdge_queues=4)
    xt_d = nc.dram_tensor("xt", [INC, SH], BF, kind="ExternalInput").ap()
    idx_d = nc.dram_tensor("idx", [128, TOTW], I32, kind="ExternalInput").ap()
    a_d = nc.dram_tensor("a_exp", [128, SH], BF, kind="ExternalInput").ap()
    dinvh_d = nc.dram_tensor("dinvh_exp", [128, SH], BF, kind="ExternalInput").ap()
    dinve_d = nc.dram_tensor("dinv_exp", [128, SH], BF, kind="ExternalInput").ap()
    w1_d = nc.dram_tensor("w1", [INC, C], BF, kind="ExternalInput").ap()
    b1r_d = nc.dram_tensor("b1r", [128, 4 * C], F32, kind="ExternalInput").ap()
    w2_d = nc.dram_tensor("w2", [C, OUTC], BF, kind="ExternalInput").ap()
    b2r_d = nc.dram_tensor("b2r", [128, OUTC], F32, kind="ExternalInput").ap()
    out_d = nc.dram_tensor("out", [SH, OUTC], F32, kind="ExternalOutput").ap()

    with tile.TileContext(nc) as tc:
        with (
            tc.tile_pool(name="dram", bufs=1, space="DRAM") as dramp,
            tc.tile_pool(name="perm", bufs=1) as perm,
        ):
            tables = []
            for kk in range(K):
                tab_k = dramp.tile(
                    [NPAD, C], BF, addr_space="Shared", name=f"table{kk}"
                )
                tables.append(tab_k)
            agin_d = dramp.tile([SH, C], BF)
            x0h_d = dramp.tile([128, SH], F32)

            acc = perm.tile([128, SH], BF)
            agin_sb = perm.tile([128, SH], BF)

            with tc.tile_pool(name="hop", bufs=1) as hop:
                a_sb = hop.tile([128, SH], BF)
                nc.sync.dma_start(out=a_sb[:], in_=a_d[:, :])
                x0d_sb = hop.tile([128, SH], BF)

                # ---------------- phase 1: h0 = x @ W1 + b1 ----------------
                with (
                    tc.tile_pool(name="p1", bufs=2) as p1,
                    tc.tile_pool(name="p1c", bufs=1) as p1c,
                    tc.tile_pool(name="ps1", bufs=2, space="PSUM") as ps1,
                ):
                    w1sb = p1c.tile([128, INC // 128, C], BF)
                    nc.sync.dma_start(
                        out=w1sb[:],
                        in_=w1_d.rearrange("(k p) c -> p k c", p=128),
                    )
                    b1r_sb = p1c.tile([128, 4 * C], F32)
                    nc.sync.dma_start(out=b1r_sb[:], in_=b1r_d[:, :])
                    dinvh1_sb = p1c.tile([128, SH], BF)
                    nc.sync.dma_start(out=dinvh1_sb[:], in_=dinvh_d[:, :])
                    dinve1_sb = p1c.tile([128, SH], BF)
                    nc.sync.dma_start(out=dinve1_sb[:], in_=dinve_d[:, :])

                    NGRP = (NB + 3) // 4
                    for g in range(NGRP):
                        nb4 = min(4, NB - 4 * g)
                        w = nb4 * 128
                        xts_list = []
                        for kc in range(4):
                            xts = p1.tile([128, 512], BF, tag=f"xts{kc}")
                            nc.sync.dma_start(
                                out=xts[:, :w],
                                in_=xt_d[
                                    kc * 128 : (kc + 1) * 128,
                                    g * 512 : g * 512 + w,
                                ],
                            )
                            xts_list.append(xts)
                        h0g = p1.tile([128, 512], F32, tag="h0g")
                        for bb in range(nb4):
                            psum = ps1.tile([128, 128], F32, tag=f"ps{bb}")
                            for kc in range(4):
                                nc.tensor.matmul(
                                    out=psum[:],
                                    lhsT=xts_list[kc][:, bb * 128 : (bb + 1) * 128],
                                    rhs=w1sb[:, kc, :],
                                    start=(kc == 0),
                                    stop=(kc == 3),
                                )
                            nc.vector.tensor_add(
                                h0g[:, bb * 128 : (bb + 1) * 128],
                                psum[:],
                                b1r_sb[:, bb * 128 : (bb + 1) * 128],
                            )
                        sl = slice(g * 512, g * 512 + w)
                        # x0h = 0.5*h0 (f32, to DRAM for the k=K blend)
                        x0hg = p1.tile([128, 512], F32, tag="x0hg")
                        nc.scalar.mul(x0hg[:, :w], h0g[:, :w], 0.5)
                        nc.sync.dma_start(out=x0h_d[:, sl], in_=x0hg[:, :w])
                        # x0d = 0.5*dinv*h0 (bf16, resident)
                        nc.vector.tensor_mul(
                            x0d_sb[:, sl], h0g[:, :w], dinvh1_sb[:, sl]
                        )
                        # ag0 = dinv*h0 (bf16)
                        nc.vector.tensor_mul(
                            agin_sb[:, sl], h0g[:, :w], dinve1_sb[:, sl]
                        )

                # ---------------- hops ----------------
                agin_rows = agin_d[:].rearrange("(b p) c -> p b c", p=128)
                agin_view = agin_sb[:].rearrange("p (b c) -> p b c", c=C)
                with (
                    tc.tile_pool(name="vals", bufs=3) as valsp,
                    tc.tile_pool(name="idxp", bufs=1) as idxp,
                ):
                    idx_sb = idxp.tile([128, TOTW], I32)
                    nc.sync.dma_start(out=idx_sb[:], in_=idx_d[:, :])
                    for k in range(1, K + 1):
                        nc.sync.dma_start(out=agin_rows, in_=agin_view)
                        table = tables[k - 1]
                        nc.gpsimd.collective_compute(
                            "AllGather",
                            mybir.AluOpType.bypass,
                            replica_groups=[list(range(NC))],
                            ins=[agin_d[:].opt()],
                            outs=[table[:, :].opt()],
                        )
                        nc.vector.memset(acc[:], 0.0)
                        from concourse.bass import IndirectOffsetOnAxis
                        for c0g, c1g, segs in groups:
                            gwid = c1g - c0g
                            vals = valsp.tile([128, GW * C], BF, tag="vals")
                            for kk in range(gwid):
                                gi = nc.gpsimd.indirect_dma_start(
                                    out=vals[:, kk * C : (kk + 1) * C],
                                    out_offset=None,
                                    in_=table[:],
                                    in_offset=IndirectOffsetOnAxis(
                                        ap=idx_sb[:, c0g + kk : c0g + kk + 1],
                                        axis=0,
                                    ),
                                )
                                q = kk % 4
                                if q:
                                    gi.ins.queue = f"qPoolDynamic{q}"
                            for jj, b0, b1_, cg in segs:
                                src_ap = vals[:, cg * C : (cg + b1_ - b0) * C]
                                dst = acc[:, b0 * C : b1_ * C]
                                nc.vector.tensor_add(dst, dst, src_ap)
                        if k < K:
                            # next table shard = A*acc + x0d
                            nc.vector.tensor_mul(agin_sb[:], acc[:], a_sb[:])
                            nc.vector.tensor_add(agin_sb[:], agin_sb[:], x0d_sb[:])

            # ---------------- phase 3: out = h5 @ W2 + b2 ----------------
            with (
                tc.tile_pool(name="p3", bufs=2) as p3,
                tc.tile_pool(name="p3c", bufs=1) as p3c,
                tc.tile_pool(name="ps3", bufs=4, space="PSUM") as ps3,
            ):
                x0h_sb = p3c.tile([128, SH], F32)
                nc.sync.dma_start(out=x0h_sb[:], in_=x0h_d[:, :])
                dinvh3_sb = p3c.tile([128, SH], BF)
                nc.sync.dma_start(out=dinvh3_sb[:], in_=dinvh_d[:, :])
                w2sb = p3c.tile([C, OUTC], BF)
                nc.sync.dma_start(out=w2sb[:], in_=w2_d[:, :])
                b2r_sb = p3c.tile([128, OUTC], F32)
                nc.sync.dma_start(out=b2r_sb[:], in_=b2r_d[:, :])
                ident = p3c.tile([128, 128], BF)
                from concourse.masks import make_identity

                make_identity(nc, ident[:])
                out_sb = p3c.tile([128, NB * OUTC], F32)

                # h5 = 0.5*dinv*acc + 0.5*h0  (bf16, into agin_sb)
                nc.vector.tensor_mul(agin_sb[:], acc[:], dinvh3_sb[:])
                nc.vector.tensor_add(agin_sb[:], agin_sb[:], x0h_sb[:])

                for b in range(NB):
                    pst = ps3.tile([128, 128], BF, tag="pst")
                    nc.tensor.transpose(
                        out=pst[:],
                        in_=agin_sb[:, b * C : (b + 1) * C],
                        identity=ident[:],
                    )
                    h5t = p3.tile([128, 128], BF, tag="h5t")
                    nc.vector.tensor_copy(out=h5t[:], in_=pst[:])
                    pso = ps3.tile([128, OUTC], F32, tag="pso")
                    nc.tensor.matmul(
                        out=pso[:],
                        lhsT=h5t[:],
                        rhs=w2sb[:],
                        start=True,
                        stop=True,
                    )
                    nc.vector.tensor_add(
                        out_sb[:, b * OUTC : (b + 1) * OUTC], pso[:], b2r_sb[:]
                    )
                nc.sync.dma_start(
                    out=out_d.rearrange("(b p) o -> p b o", p=128),
                    in_=out_sb[:].rearrange("p (b o) -> p b o", o=OUTC),
                )

    nc.compile()
    return nc


def _run(inputs, trace=False):
    in_maps, newid, meta = _preprocess(**inputs)
    nc = _build(meta)
    res = run_bass_kernel_spmd(
        nc, in_maps, core_ids=list(range(NC)), trace=trace
    )
    full = np.concatenate([res.results[c]["out"] for c in range(NC)], axis=0)
    out = full[newid].astype(np.float32)
    return out, res.exec_time_ns


def kernel(**inputs) -> np.ndarray:
    trace = bool(int(os.environ.get("APPNP_TRACE", "0")))
    out, _ = _run(inputs, trace=trace)
    return out


if __name__ == "__main__":
    rng = np.random.default_rng(0)
    demo = {
        "x": rng.standard_normal((N, INC), dtype=np.float32),
        "edge_index": rng.integers(0, N, size=(2, 1600000)).astype(np.int64),
        "W1": rng.standard_normal((INC, C), dtype=np.float32) / 22.6,
        "b1": rng.standard_normal(C, dtype=np.float32) * 0.01,
        "W2": rng.standard_normal((C, OUTC), dtype=np.float32) / 11.3,
        "b2": rng.standard_normal(OUTC, dtype=np.float32) * 0.01,
    }
    out, ns = _run(demo, trace=True)
    print("exec_time_ns:", ns, "out", out.shape, out.dtype)


# revision 13
# speedup vs baseline: 1.1656x; 1.0003x over previous
"""APPNP GNN (GCN-normalized propagation, K=5, alpha=0.5) on 8 TRN2 NeuronCores.

Strategy:
  - Nodes relabeled: dealt round-robin by degree to 8 cores (balanced shards),
    then ordered within each core by max-per-range in-edge count so that each
    128-dest partition-block has near-uniform per-range slot needs.
  - h staged as a bf16 [100352, 128] table replicated in each core's DRAM,
    re-assembled per hop via AllGather of the 8 shard updates (one Shared
    table per hop: single-writer rule).
  - Per hop each core gathers its edges' source rows with gpsimd.dma_gather
    (int16 indices into 25088-row windows = core-pair ranges; dest-slot
    layout: partition = dest-in-block, column = (range, slot j, block)),
    accumulating slots into a bf16 accumulator on VectorE.
  - GCN norm folded into per-node scalings: table rows are dinv*h and the
    per-dest dinv folds into A = 0.5*dinv^2 at the blend:
        next_table = A * acc + 0.5*dinv*h0
  - lin1/lin2 run on TensorE in bf16.
"""

import os
import sys
import types

import numpy as np

sys.path.insert(0, "/opt/trn_rl_repo")


def _install_ntff_shim():
    """antenv.axon_hooks is absent in this image; concourse imports it when
    trace=True under axon. Provide it, backed by the boot's ctypes hook."""
    if "antenv.axon_hooks" in sys.modules:
        return
    mod = types.ModuleType("antenv.axon_hooks")
    mod._hook = None

    def set_axon_ntff_profile_hook(h):
        mod._hook = h

    def get_axon_ntff_profile_hook():
        if mod._hook is None:
            try:
                from trn_agent_boot.trn_boot import _ntff_profile_via_ctypes

                mod._hook = _ntff_profile_via_ctypes("/opt/axon/libaxon_pjrt.so")
            except Exception:
                return None
        return mod._hook

    mod.set_axon_ntff_profile_hook = set_axon_ntff_profile_hook
    mod.get_axon_ntff_profile_hook = get_axon_ntff_profile_hook
    sys.modules["antenv.axon_hooks"] = mod


_install_ntff_shim()

import ml_dtypes  # noqa: E402

import concourse.tile as tile  # noqa: E402
from concourse import bacc, mybir  # noqa: E402
from concourse.bass_utils import run_bass_kernel_spmd  # noqa: E402

N = 100000
NC = 8
C = 128
INC = 512
OUTC = 40
K = 5
NB = 98
SH = NB * 128          # 12544 dest slots per core
NPAD = NC * SH         # 100352
# position 12500 of core 0 is a pad node (dinv=0): its table row is always 0,
# so it doubles as the zero row for padded gather slots.
ZROW = 12500
GW = 98                # gather columns per vals tile

BF = mybir.dt.bfloat16
F32 = mybir.dt.float32
I32 = mybir.dt.int32

DT_NS = None


def _bf16(a):
    return np.ascontiguousarray(a.astype(ml_dtypes.bfloat16))


def _preprocess(x, edge_index, W1, b1, W2, b2):
    x = np.asarray(x, dtype=np.float32)
    ei = np.asarray(edge_index).astype(np.int64)
    W1 = np.asarray(W1, dtype=np.float32)
    b1 = np.asarray(b1, dtype=np.float32)
    W2 = np.asarray(W2, dtype=np.float32)
    b2 = np.asarray(b2, dtype=np.float32)

    loops = np.arange(N, dtype=np.int64)
    row = np.concatenate([ei[0], loops])
    col = np.concatenate([ei[1], loops])
    deg = np.bincount(col, minlength=N).astype(np.int64)
    dinv = np.where(deg > 0, 1.0 / np.sqrt(deg.astype(np.float64)), 0.0)

    # relabel: deal nodes round-robin by degree to the 8 cores, positions by
    # degree desc within each core (block slot-counts then decrease with b)
    order = np.argsort(-deg, kind="stable")
    i = np.arange(N)
    newid = np.empty(N, dtype=np.int64)
    newid[order] = (i % NC) * SH + (i // NC)

    rn = newid[row]
    cn = newid[col]

    # slot index j within each destination
    s = np.argsort(cn, kind="stable")
    cn_s = cn[s]
    rn_s = rn[s]
    cnt = np.bincount(cn_s, minlength=NC * SH)
    starts = np.zeros(NC * SH, dtype=np.int64)
    np.cumsum(cnt[:-1], out=starts[1:])
    j_s = np.arange(len(cn_s)) - starts[cn_s]

    # unified per-block max degree across cores, monotone over blocks
    Lb = cnt.reshape(NC, NB, 128).max(axis=(0, 2)).astype(np.int64)
    Lb = np.maximum.accumulate(Lb[::-1])[::-1]
    maxL = int(Lb.max())
    nb_j = np.array([(Lb > j).sum() for j in range(maxL)], dtype=np.int64)
    off = np.zeros(maxL + 1, dtype=np.int64)
    np.cumsum(nb_j, out=off[1:])
    TOTW = int(off[-1])

    # idx tensors [NC, 128, TOTW] int32 (global row ids)
    idx = np.full((NC, 128, TOTW), ZROW, dtype=np.int32)
    core_s = cn_s // SH
    dloc = cn_s % SH
    b_s = dloc // 128
    p_s = dloc % 128
    colpos = off[j_s] + b_s
    idx[core_s, p_s, colpos] = rn_s.astype(np.int32)

    # column groups of <=GW with per-j acc-add segments
    groups = []  # (c0, c1, [(j, b0, b1, col_in_group)])
    c0 = 0
    while c0 < TOTW:
        c1 = min(c0 + GW, TOTW)
        segs = []
        for j in range(maxL):
            lo = max(int(off[j]), c0)
            hi = min(int(off[j + 1]), c1)
            if lo < hi:
                segs.append((j, lo - int(off[j]), hi - int(off[j]), lo - c0))
        groups.append((c0, c1, segs))
        c0 = c1
    meta = dict(TOTW=TOTW, groups=groups)

    # per-core node data in new-id space
    xfull = np.zeros((NC * SH, INC), dtype=np.float32)
    xfull[newid] = x
    dinvN = np.zeros(NC * SH, dtype=np.float32)
    dinvN[newid] = dinv.astype(np.float32)

    in_maps = []
    w1_bf = _bf16(W1)
    b1r = np.ascontiguousarray(np.tile(b1[None, :], (128, 4)).astype(np.float32))
    w2_bf = _bf16(W2)
    b2r = np.ascontiguousarray(np.tile(b2[None, :], (128, 1)).astype(np.float32))
    for c in range(NC):
        dv = dinvN[c * SH : (c + 1) * SH].reshape(NB, 128).T  # [128, NB]
        a_pb = (0.5 * dv * dv).astype(np.float32)
        dh_pb = (0.5 * dv).astype(np.float32)
        a_exp = np.repeat(a_pb[:, :, None], C, axis=2).reshape(128, NB * C)
        dinvh_exp = np.repeat(dh_pb[:, :, None], C, axis=2).reshape(128, NB * C)
        dinv_exp = np.repeat(dv[:, :, None].astype(np.float32), C, axis=2).reshape(
            128, NB * C
        )
        in_maps.append(
            {
                "xt": _bf16(xfull[c * SH : (c + 1) * SH].T),
                "idx": np.ascontiguousarray(idx[c]),
                "a_exp": _bf16(a_exp),
                "dinvh_exp": _bf16(dinvh_exp),
                "dinv_exp": _bf16(dinv_exp),
                "w1": w1_bf,
                "b1r": b1r,
                "w2": w2_bf,
                "b2r": b2r,
            }
        )
    return in_maps, newid, meta


def _build(meta):
    TOTW = meta["TOTW"]
    groups = meta["groups"]

    nc = bacc.Bacc("TRN2", target_bir_lowering=False, debug=False, num_devices=NC,
                   num_swdge_queues=4)
    xt_d = nc.dram_tensor("xt", [INC, SH], BF, kind="ExternalInput").ap()
    idx_d = nc.dram_tensor("idx", [128, TOTW], I32, kind="ExternalInput").ap()
    a_d = nc.dram_tensor("a_exp", [128, SH], BF, kind="ExternalInput").ap()
    dinvh_d = nc.dram_tensor("dinvh_exp", [128, SH], BF, kind="ExternalInput").ap()
    dinve_d = nc.dram_tensor("dinv_exp", [128, SH], BF, kind="ExternalInput").ap()
    w1_d = nc.dram_tensor("w1", [INC, C], BF, kind="ExternalInput").ap()
    b1r_d = nc.dram_tensor("b1r", [128, 4 * C], F32, kind="ExternalInput").ap()
    w2_d = nc.dram_tensor("w2", [C, OUTC], BF, kind="ExternalInput").ap()
    b2r_d = nc.dram_tensor("b2r", [128, OUTC], F32, kind="ExternalInput").ap()
    out_d = nc.dram_tensor("out", [SH, OUTC], F32, kind="ExternalOutput").ap()

    with tile.TileContext(nc) as tc:
        with (
            tc.tile_pool(name="dram", bufs=1, space="DRAM") as dramp,
            tc.tile_pool(name="perm", bufs=1) as perm,
        ):
            tables = []
            for kk in range(K):
                tab_k = dramp.tile(
                    [NPAD, C], BF, addr_space="Shared", name=f"table{kk}"
                )
                tables.append(tab_k)
            agin_d = dramp.tile([SH, C], BF)
            x0h_d = dramp.tile([128, SH], F32)

            acc = perm.tile([128, SH], BF)
            agin_sb = perm.tile([128, SH], BF)

            with tc.tile_pool(name="hop", bufs=1) as hop:
                a_sb = hop.tile([128, SH], BF)
                nc.sync.dma_start(out=a_sb[:], in_=a_d[:, :])
                x0d_sb = hop.tile([128, SH], BF)

                # ---------------- phase 1: h0 = x @ W1 + b1 ----------------
                with (
                    tc.tile_pool(name="p1", bufs=2) as p1,
                    tc.tile_pool(name="p1c", bufs=1) as p1c,
                    tc.tile_pool(name="ps1", bufs=2, space="PSUM") as ps1,
                ):
                    w1sb = p1c.tile([128, INC // 128, C], BF)
                    nc.sync.dma_start(
                        out=w1sb[:],
                        in_=w1_d.rearrange("(k p) c -> p k c", p=128),
                    )
                    b1r_sb = p1c.tile([128, 4 * C], F32)
                    nc.sync.dma_start(out=b1r_sb[:], in_=b1r_d[:, :])
                    dinvh1_sb = p1c.tile([128, SH], BF)
                    nc.sync.dma_start(out=dinvh1_sb[:], in_=dinvh_d[:, :])
                    dinve1_sb = p1c.tile([128, SH], BF)
                    nc.sync.dma_start(out=dinve1_sb[:], in_=dinve_d[:, :])

                    NGRP = (NB + 3) // 4
                    for g in range(NGRP):
                        nb4 = min(4, NB - 4 * g)
                        w = nb4 * 128
                        xts_list = []
                        for kc in range(4):
                            xts = p1.tile([128, 512], BF, tag=f"xts{kc}")
                            nc.sync.dma_start(
                                out=xts[:, :w],
                                in_=xt_d[
                                    kc * 128 : (kc + 1) * 128,
                                    g * 512 : g * 512 + w,
                                ],
                            )
                            xts_list.append(xts)
                        h0g = p1.tile([128, 512], F32, tag="h0g")
                        for bb in range(nb4):
                            psum = ps1.tile([128, 128], F32, tag=f"ps{bb}")
                            for kc in range(4):
                                nc.tensor.matmul(
                                    out=psum[:],
                                    lhsT=xts_list[kc][:, bb * 128 : (bb + 1) * 128],
                                    rhs=w1sb[:, kc, :],
                                    start=(kc == 0),
                                    stop=(kc == 3),
                                )
                            nc.vector.tensor_add(
                                h0g[:, bb * 128 : (bb + 1) * 128],
                                psum[:],
                                b1r_sb[:, bb * 128 : (bb + 1) * 128],
                            )
                        sl = slice(g * 512, g * 512 + w)
                        # x0h = 0.5*h0 (f32, to DRAM for the k=K blend)
                        x0hg = p1.tile([128, 512], F32, tag="x0hg")
                        nc.scalar.mul(x0hg[:, :w], h0g[:, :w], 0.5)
                        nc.sync.dma_start(out=x0h_d[:, sl], in_=x0hg[:, :w])
                        # x0d = 0.5*dinv*h0 (bf16, resident)
                        nc.vector.tensor_mul(
                            x0d_sb[:, sl], h0g[:, :w], dinvh1_sb[:, sl]
                        )
                        # ag0 = dinv*h0 (bf16)
                        nc.vector.tensor_mul(
                            agin_sb[:, sl], h0g[:, :w], dinve1_sb[:, sl]
                        )

                # ---------------- hops ----------------
                agin_rows = agin_d[:].rearrange("(b p) c -> p b c", p=128)
                agin_view = agin_sb[:].rearrange("p (b c) -> p b c", c=C)
                with (
                    tc.tile_pool(name="vals", bufs=4) as valsp,
                    tc.tile_pool(name="idxp", bufs=1) as idxp,
                ):
                    idx_sb = idxp.tile([128, TOTW], I32)
                    nc.sync.dma_start(out=idx_sb[:], in_=idx_d[:, :])
                    for k in range(1, K + 1):
                        nc.sync.dma_start(out=agin_rows, in_=agin_view)
                        table = tables[k - 1]
                        nc.gpsimd.collective_compute(
                            "AllGather",
                            mybir.AluOpType.bypass,
                            replica_groups=[list(range(NC))],
                            ins=[agin_d[:].opt()],
                            outs=[table[:, :].opt()],
                        )
                        nc.vector.memset(acc[:], 0.0)
                        from concourse.bass import IndirectOffsetOnAxis
                        for c0g, c1g, segs in groups:
                            gwid = c1g - c0g
                            vals = valsp.tile([128, GW * C], BF, tag="vals")
                            for kk in range(gwid):
                                gi = nc.gpsimd.indirect_dma_start(
                                    out=vals[:, kk * C : (kk + 1) * C],
                                    out_offset=None,
                                    in_=table[:],
                                    in_offset=IndirectOffsetOnAxis(
                                        ap=idx_sb[:, c0g + kk : c0g + kk + 1],
                                        axis=0,
                                    ),
                                )
                                q = kk % 4
                                if q:
                                    gi.ins.queue = f"qPoolDynamic{q}"
                            for jj, b0, b1_, cg in segs:
                                src_ap = vals[:, cg * C : (cg + b1_ - b0) * C]
                                dst = acc[:, b0 * C : b1_ * C]
                                nc.vector.tensor_add(dst, dst, src_ap)
                        if k < K:
                            # next table shard = A*acc + x0d
                            nc.vector.tensor_mul(agin_sb[:], acc[:], a_sb[:])
                            nc.vector.tensor_add(agin_sb[:], agin_sb[:], x0d_sb[:])

            # ---------------- phase 3: out = h5 @ W2 + b2 ----------------
            with (
                tc.tile_pool(name="p3", bufs=2) as p3,
                tc.tile_pool(name="p3c", bufs=1) as p3c,
                tc.tile_pool(name="ps3", bufs=4, space="PSUM") as ps3,
            ):
                x0h_sb = p3c.tile([128, SH], F32)
                nc.sync.dma_start(out=x0h_sb[:], in_=x0h_d[:, :])
                dinvh3_sb = p3c.tile([128, SH], BF)
                nc.sync.dma_start(out=dinvh3_sb[:], in_=dinvh_d[:, :])
                w2sb = p3c.tile([C, OUTC], BF)
                nc.sync.dma_start(out=w2sb[:], in_=w2_d[:, :])
                b2r_sb = p3c.tile([128, OUTC], F32)
                nc.sync.dma_start(out=b2r_sb[:], in_=b2r_d[:, :])
                ident = p3c.tile([128, 128], BF)
                from concourse.masks import make_identity

                make_identity(nc, ident[:])
                out_sb = p3c.tile([128, NB * OUTC], F32)

                # h5 = 0.5*dinv*acc + 0.5*h0  (bf16, into agin_sb)
                nc.vector.tensor_mul(agin_sb[:], acc[:], dinvh3_sb[:])
                nc.vector.tensor_add(agin_sb[:], agin_sb[:], x0h_sb[:])

                for b in range(NB):
                    pst = ps3.tile([128, 128], BF, tag="pst")
                    nc.tensor.transpose(
                        out=pst[:],
                        in_=agin_sb[:, b * C : (b + 1) * C],
                        identity=ident[:],
                    )
                    h5t = p3.tile([128, 128], BF, tag="h5t")
                    nc.vector.tensor_copy(out=h5t[:], in_=pst[:])
                    pso = ps3.tile([128, OUTC], F32, tag="pso")
                    nc.tensor.matmul(
                        out=pso[:],
                        lhsT=h5t[:],
                        rhs=w2sb[:],
                        start=True,
                        stop=True,
                    )
                    nc.vector.tensor_add(
                        out_sb[:, b * OUTC : (b + 1) * OUTC], pso[:], b2r_sb[:]
                    )
                nc.sync.dma_start(
                    out=out_d.rearrange("(b p) o -> p b o", p=128),
                    in_=out_sb[:].rearrange("p (b o) -> p b o", o=OUTC),
                )

    nc.compile()
    return nc


def _run(inputs, trace=False):
    in_maps, newid, meta = _preprocess(**inputs)
    nc = _build(meta)
    res = run_bass_kernel_spmd(
        nc, in_maps, core_ids=list(range(NC)), trace=trace
    )
    full = np.concatenate([res.results[c]["out"] for c in range(NC)], axis=0)
    out = full[newid].astype(np.float32)
    return out, res.exec_time_ns


def kernel(**inputs) -> np.ndarray:
    trace = bool(int(os.environ.get("APPNP_TRACE", "0")))
    out, _ = _run(inputs, trace=trace)
    return out


if __name__ == "__main__":
    rng = np.random.default_rng(0)
    demo = {
        "x": rng.standard_normal((N, INC), dtype=np.float32),
        "edge_index": rng.integers(0, N, size=(2, 1600000)).astype(np.int64),
        "W1": rng.standard_normal((INC, C), dtype=np.float32) / 22.6,
        "b1": rng.standard_normal(C, dtype=np.float32) * 0.01,
        "W2": rng.standard_normal((C, OUTC), dtype=np.float32) / 11.3,
        "b2": rng.standard_normal(OUTC, dtype=np.float32) * 0.01,
    }
    out, ns = _run(demo, trace=True)
    print("exec_time_ns:", ns, "out", out.shape, out.dtype)
